# revision 33
# baseline (speedup 1.0000x reference)
"""Trainium2 Bass kernel for a dense transformer encoder layer.

Reference semantics (B=2, S=2048, D=1024, H=16, DH=64, HID=4096):
    q = einsum('bsd,hde->bhse', x, Wq) + bq          (q == k == v, source bug)
    prob = softmax(q @ q^T / sqrt(DH))
    attn = concat_heads(prob @ q)
    x1 = LN(x + attn);  ff = relu(x1 @ W1 + b1) @ W2 + b2;  out = LN(x1 + ff)

Sharding: 8 cores, core c -> batch b=c//4, token quarter t=c%4.  Each core
computes q for the full sequence of its batch (replicated inside the 4-core
group -> zero collectives), then attention + FFN for its own 512 tokens.
The host rotates each core's copy of x[b] so its quarter lands at rows 0:512
(attention is permutation-equivariant over keys), and reassembles quarters.

On-chip dataflow is bf16 matmul / f32 accumulate. Layout flips (x->xT,
qT->q-natural(+ones column for softmax denominators), uvT->attn, x1->x1T,
ffT->ff) go through DRAM round trips using the HWDGE xbar transpose.
Scratch tensors are split into head/d halves so the post-attention and
post-FFN epilogues start while the second half is still computing; epilogue
pools are opened before the attention/FFN pools so their SBUF regions are
disjoint (stack reuse would otherwise serialize the phases).
"""

import numpy as np

import concourse.bacc as bacc
import concourse.mybir as mybir
from concourse import tile
from concourse.bass_utils import run_bass_kernel_spmd

dt = mybir.dt
AF = mybir.ActivationFunctionType
ALU = mybir.AluOpType

B, S, D = 2, 2048, 1024
H, DH, HID = 16, 64, 256 * 16
SQ = S // 4            # tokens per core
NCORES = 8
EPS = 1e-5
F32, BF16 = dt.float32, dt.bfloat16

_BUILD_CACHE = {}


def _build(apply_affine: bool):
    if apply_affine in _BUILD_CACHE:
        return _BUILD_CACHE[apply_affine]

    nc = bacc.Bacc("TRN2", target_bir_lowering=False, debug=False,
                   num_devices=NCORES)

    x_bf = nc.dram_tensor("x_bf", [S, D], BF16, kind="ExternalInput").ap()
    x_q = nc.dram_tensor("x_q", [SQ, D], F32, kind="ExternalInput").ap()
    wq = nc.dram_tensor("wq", [D, D], BF16, kind="ExternalInput").ap()
    bq_r = nc.dram_tensor("bq_r", [128, 8], F32, kind="ExternalInput").ap()
    w1 = nc.dram_tensor("w1", [D, HID], BF16, kind="ExternalInput").ap()
    b1_r = nc.dram_tensor("b1_r", [128, 32], F32, kind="ExternalInput").ap()
    w2 = nc.dram_tensor("w2", [HID, D], BF16, kind="ExternalInput").ap()
    b2_r = nc.dram_tensor("b2_r", [128, 8], F32, kind="ExternalInput").ap()
    if apply_affine:
        g1d = nc.dram_tensor("g1d", [128, D], F32, kind="ExternalInput").ap()
        be1d = nc.dram_tensor("be1d", [128, D], F32, kind="ExternalInput").ap()
        g2d = nc.dram_tensor("g2d", [128, D], F32, kind="ExternalInput").ap()
        be2d = nc.dram_tensor("be2d", [128, D], F32, kind="ExternalInput").ap()
    out_q = nc.dram_tensor("out_q", [SQ, D], F32, kind="ExternalOutput").ap()

    with tile.TileContext(nc) as tc:
        with (
            tc.tile_pool(name="dram", bufs=1, space="DRAM") as dpool,
            tc.tile_pool(name="const", bufs=1) as cpool,
        ):
            q_d = dpool.tile([H * 80, S], BF16)
            uv_dl = dpool.tile([D // 2, SQ], BF16)
            uv_dh = dpool.tile([D // 2, SQ], BF16)
            cs_dl = dpool.tile([4, 1024], F32)
            cs_dh = dpool.tile([4, 1024], F32)
            ff_dl = dpool.tile([D // 2, SQ], BF16)
            ff_dh = dpool.tile([D // 2, SQ], BF16)
            cs16l = cs_dl.rearrange("a (j s) -> (a j) s", j=2)
            cs16h = cs_dh.rearrange("a (j s) -> (a j) s", j=2)

            bq_sb = cpool.tile([128, 8], F32)
            nc.scalar.dma_start(bq_sb[:], bq_r[:])
            b1_sb = cpool.tile([128, 32], F32)
            nc.scalar.dma_start(b1_sb[:], b1_r[:])
            b2_sb = cpool.tile([128, 8], F32)
            nc.scalar.dma_start(b2_sb[:], b2_r[:])
            if apply_affine:
                g1_sb = cpool.tile([128, D], F32)
                nc.scalar.dma_start(g1_sb[:], g1d[:])
                be1_sb = cpool.tile([128, D], F32)
                nc.scalar.dma_start(be1_sb[:], be1d[:])
                g2_sb = cpool.tile([128, D], F32)
                nc.scalar.dma_start(g2_sb[:], g2d[:])
                be2_sb = cpool.tile([128, D], F32)
                nc.scalar.dma_start(be2_sb[:], be2d[:])

            eps_sb = cpool.tile([128, 1], F32)
            nc.vector.memset(eps_sb[:], EPS)

            # 128x128 bf16 identity for PE-mode transposes
            col_i = cpool.tile([128, 128], F32)
            nc.gpsimd.iota(col_i[:], [[1, 128]], channel_multiplier=0,
                           allow_small_or_imprecise_dtypes=True)
            row_i = cpool.tile([128, 1], F32)
            nc.gpsimd.iota(row_i[:], [[0, 1]], channel_multiplier=1,
                           allow_small_or_imprecise_dtypes=True)
            idn = cpool.tile([128, 128], BF16)
            nc.vector.tensor_scalar(idn[:], col_i[:], row_i[:, 0:1], None,
                                    ALU.is_equal)

            # ones rows of the augmented q (row 64 of every 80-row head block)
            ones_bf = cpool.tile([16, 512], BF16)
            nc.vector.memset(ones_bf[:], 1.0)
            q_d_rows = q_d.rearrange("(h r) s -> h r s", r=80)
            for c4 in range(4):
                nc.scalar.dma_start(
                    q_d_rows[:, 64, c4 * 512:(c4 + 1) * 512], ones_bf[:])

            with (
                tc.tile_pool(name="qT", bufs=1) as qTpool,
                tc.tile_pool(name="ln1", bufs=2) as lpool,
                tc.tile_pool(name="x1f", bufs=1) as x1pool,
                tc.tile_pool(name="ln2", bufs=2) as l2pool,
                tc.tile_pool(name="x1T", bufs=1) as xtp,
            ):
                x1T = [xtp.tile([128, SQ], BF16, tag=f"x1T{k}",
                                name=f"x1T{k}") for k in range(8)]
                qT = [qTpool.tile([128, S], BF16, tag=f"qT{e}", name=f"qT{e}")
                      for e in range(8)]

                # Phase-D input loaders; emitted mid-attention so the first
                # half streams in while heads 8-15 are still computing.
                ln_in = [[None] * 4, [None] * 4]   # [half][sub] -> (rct, ab)
                y1s = []

                def emit_ln1_inputs(hi, subs=range(4)):
                    cs16x = (cs16l, cs16h)[hi]
                    uv_dx = (uv_dl, uv_dh)[hi]
                    for sub in subs:
                        scols = slice(sub * 128, (sub + 1) * 128)
                        ct = lpool.tile([128, 8], F32, tag=f"ct{hi}", bufs=4,
                                        name=f"ct{hi}_{sub}")
                        nc.sync.dma_start(
                            ct[:], cs16x[:, scols].rearrange("h p -> p h"))
                        rct = lpool.tile([128, 8], F32, tag=f"rct{hi}", bufs=4,
                                         name=f"rct{hi}_{sub}")
                        nc.vector.reciprocal(rct[:], ct[:])
                        ab = lpool.tile([128, D // 2], BF16, tag=f"attn{hi}",
                                        bufs=4, name=f"attn{hi}_{sub}")
                        nc.sync.dma_start(ab[:], uv_dx[:, scols],
                                          transpose=True)
                        ln_in[hi][sub] = (rct, ab)
                        if hi == 0:
                            y1 = lpool.tile([128, D], F32, tag="y1", bufs=4,
                                            name=f"y1_{sub}")
                            nc.scalar.dma_start(
                                y1[:], x_q[sub * 128:(sub + 1) * 128, :])
                            y1s.append(y1)

                # ---- Phases B+C interleaved: qproj(e=p) then attention
                # pair p; the next pair's projection fills PE while ACT burns
                # through the softmax exps of the current pair. ----
                with (
                    tc.tile_pool(name="xT", bufs=1) as xTpool,
                    tc.tile_pool(name="wq", bufs=1) as wqpool,
                    tc.tile_pool(name="qa", bufs=1) as qapool,
                    tc.tile_pool(name="att", bufs=4) as apool,
                    tc.tile_pool(name="qps", bufs=2, space="PSUM") as qps,
                    tc.tile_pool(name="scps", bufs=2, space="PSUM") as scps,
                    tc.tile_pool(name="uvps", bufs=2, space="PSUM") as uvps,
                ):
                    xT = []
                    for k in range(8):
                        cols = slice(k * 128, (k + 1) * 128)
                        t = xTpool.tile([128, S], BF16, tag=f"xT{k}")
                        eng = nc.sync if k % 2 == 0 else nc.scalar
                        eng.dma_start(t[:], x_bf[:, cols], transpose=True)
                        xT.append(t)
                    wq_sb = []
                    for k in range(8):
                        t = wqpool.tile([128, D], BF16, tag=f"wq{k}")
                        nc.scalar.dma_start(t[:], wq[k * 128:(k + 1) * 128, :])
                        wq_sb.append(t)

                    for p in range(8):
                        # q projection for head pair p -> qT[p]
                        for n in range(4):
                            ps = qps.tile([128, 512], F32, tag="qps",
                                          name=f"qps{p}_{n}")
                            for k in range(8):
                                nc.tensor.matmul(
                                    ps[:],
                                    wq_sb[k][:, p * 128:(p + 1) * 128],
                                    xT[k][:, n * 512:(n + 1) * 512],
                                    start=(k == 0), stop=(k == 7))
                            nc.vector.tensor_scalar_add(
                                qT[p][:, n * 512:(n + 1) * 512], ps[:],
                                bq_sb[:, p:p + 1])
                        # store into q_d (80-row head blocks; row 64 is ones)
                        for half in range(2):
                            h = 2 * p + half
                            nc.sync.dma_start(
                                q_d[h * 80:h * 80 + 64, :],
                                qT[p][half * 64:half * 64 + 64, :])
                        # natural-layout augmented q for this pair
                        qa_p = []
                        for c in range(16):
                            t = qapool.tile([128, 160], BF16, tag=f"qa{c % 4}",
                                            bufs=8, name=f"qa{p}_{c}")
                            nc.sync.dma_start(
                                t[:],
                                q_d[p * 160:(p + 1) * 160,
                                    c * 128:(c + 1) * 128],
                                transpose=True)
                            qa_p.append(t)

                        # attention for heads 2p, 2p+1
                        uv = [uvps.tile([65, 512], F32, tag="uv",
                                        name=f"uv{p}_{i}") for i in range(2)]
                        prev = None  # (E0, E1, cg)

                        def emit_wv(E0p, E1p, cgp, start, stop):
                            for cc in range(2):
                                c = 2 * cgp + cc
                                for half, Ep in ((0, E0p), (1, E1p)):
                                    nc.tensor.matmul(
                                        uv[half][:],
                                        qa_p[c][:, half * 80:half * 80 + 65],
                                        Ep[:, cc * 512:(cc + 1) * 512],
                                        start=start and cc == 0,
                                        stop=stop and cc == 1)

                        for cg in range(8):
                            sc = [scps.tile([128, 1024], F32, tag="sc",
                                            name=f"sc{p}_{cg}_{i}")
                                  for i in range(2)]
                            for cc in range(2):
                                c = 2 * cg + cc
                                for half in range(2):
                                    nc.tensor.matmul(
                                        sc[half][:, cc * 512:(cc + 1) * 512],
                                        qT[p][half * 64:half * 64 + 64,
                                              c * 128:(c + 1) * 128],
                                        qT[p][half * 64:half * 64 + 64,
                                              0:512],
                                        start=True, stop=True)
                            E = [apool.tile([128, 1024], BF16, tag="E",
                                            bufs=3, name=f"E{p}_{cg}_{i}")
                                 for i in range(2)]
                            for half in range(2):
                                nc.scalar.activation(
                                    E[half][:], sc[half][:], AF.Exp,
                                    scale=0.125)
                            if prev is not None:
                                emit_wv(prev[0], prev[1], prev[2],
                                        prev[2] == 0, False)
                            prev = (E[0], E[1], cg)
                        emit_wv(prev[0], prev[1], prev[2], False, True)

                        # unnormalized head outputs + softmax denominators
                        uv_dst = uv_dl if p < 4 else uv_dh
                        cs_dst = cs_dl if p < 4 else cs_dh
                        pp = p % 4
                        for half in range(2):
                            h = 2 * pp + half
                            uvT_sb = apool.tile([64, 512], BF16, tag="uvT")
                            nc.vector.tensor_copy(uvT_sb[:],
                                                  uv[half][0:64, :])
                            nc.sync.dma_start(
                                uv_dst[h * 64:(h + 1) * 64, :], uvT_sb[:])
                            cs_sb = apool.tile([65, 512], F32, tag="cs",
                                               bufs=2,
                                               name=f"cs{p}_{half}")
                            nc.vector.tensor_copy(cs_sb[64:65, :],
                                                  uv[half][64:65, :])
                            nc.sync.dma_start(
                                cs_dst[pp:pp + 1,
                                       half * 512:(half + 1) * 512],
                                cs_sb[64:65, :])
                        if p >= 3:
                            # stream the heads-0..7 epilogue one token-sub at
                            # a time behind pairs 3-6 to spread DVE load
                            sub = p - 3
                            if sub < 4:
                                emit_ln1_inputs(0, [sub])
                                rct, ab = ln_in[0][sub]
                                for hh in range(8):
                                    sl = slice(hh * 64, (hh + 1) * 64)
                                    nc.vector.scalar_tensor_tensor(
                                        y1s[sub][:, sl],
                                        ab[:, hh * 64:(hh + 1) * 64],
                                        rct[:, hh:hh + 1],
                                        y1s[sub][:, sl], ALU.mult, ALU.add)
                    emit_ln1_inputs(1)

                # ---- Phase D: heads 8-15 residual + LN1 ----
                for sub in range(4):
                    rct, ab = ln_in[1][sub]
                    for hh in range(8):
                        h = 8 + hh
                        sl = slice(h * 64, (h + 1) * 64)
                        nc.vector.scalar_tensor_tensor(
                            y1s[sub][:, sl],
                            ab[:, hh * 64:(hh + 1) * 64],
                            rct[:, hh:hh + 1],
                            y1s[sub][:, sl], ALU.mult, ALU.add)
                x1_f32 = []
                with (
                    tc.tile_pool(name="lnps", bufs=2, space="PSUM") as lnps,
                    tc.tile_pool(name="tps", bufs=4, space="PSUM") as tps,
                ):
                    for sub in range(4):
                        x1 = x1pool.tile([128, D], F32, tag=f"x1_{sub}",
                                         name=f"x1_{sub}")
                        _layer_norm(nc, lpool, lnps, y1s[sub], x1, eps_sb,
                                    (g1_sb, be1_sb) if apply_affine
                                    else None)
                        x1_f32.append(x1)
                        x1bf = lpool.tile([128, D], BF16, tag="x1bf",
                                          bufs=2)
                        nc.scalar.copy(x1bf[:], x1[:])
                        for k in range(8):
                            pst = tps.tile([128, 128], BF16, tag="tps",
                                           name=f"tps{sub}_{k}")
                            nc.tensor.transpose(
                                pst[:], x1bf[:, k * 128:(k + 1) * 128],
                                idn[:])
                            nc.scalar.copy(
                                x1T[k][:, sub * 128:(sub + 1) * 128],
                                pst[:])

                ffb_lo = [None] * 4
                y2s = []

                def emit_ffb(hi, store):
                    ff_dx = (ff_dl, ff_dh)[hi]
                    for sub in range(4):
                        scols = slice(sub * 128, (sub + 1) * 128)
                        ffb = l2pool.tile([128, D // 2], BF16,
                                          tag=f"ffb{hi}", bufs=4,
                                          name=f"ffb{hi}_{sub}")
                        nc.sync.dma_start(ffb[:], ff_dx[:, scols],
                                          transpose=True)
                        store[sub] = ffb

                # ---- Phase E: FFN ----
                with (
                    tc.tile_pool(name="h1", bufs=1) as h1pool,
                    tc.tile_pool(name="wstr", bufs=3) as wpool,
                    tc.tile_pool(name="fps", bufs=4, space="PSUM") as fps,
                ):
                    h1t = []
                    for j in range(32):
                        w1t = wpool.tile([128, 8, 128], BF16, tag="w1t",
                                         bufs=5)
                        nc.scalar.dma_start(
                            w1t[:],
                            w1[:, j * 128:(j + 1) * 128]
                            .rearrange("(k p) c -> p k c", p=128))
                        ps = fps.tile([128, 512], F32, tag="fps")
                        for k in range(8):
                            nc.tensor.matmul(ps[:], w1t[:, k, :], x1T[k][:],
                                             start=(k == 0), stop=(k == 7))
                        ht = h1pool.tile([128, SQ], BF16, tag=f"h1_{j}")
                        nc.vector.tensor_scalar(
                            ht[:], ps[:], b1_sb[:, j:j + 1], 0.0,
                            ALU.add, ALU.max)
                        h1t.append(ht)
                    w2r = w2.rearrange("(j p) c -> j p c", p=128)
                    for i in range(8):
                        w2h = []
                        for hh in range(2):
                            t = wpool.tile([128, 16, 128], BF16, tag="w2t",
                                           bufs=4, name=f"w2t{i}_{hh}")
                            nc.scalar.dma_start(
                                t[:],
                                w2r[hh * 16:(hh + 1) * 16, :,
                                    i * 128:(i + 1) * 128]
                                .rearrange("j p c -> p j c"))
                            w2h.append(t)
                        ps = fps.tile([128, 512], F32, tag="fps")
                        for j in range(32):
                            nc.tensor.matmul(ps[:], w2h[j // 16][:, j % 16, :],
                                             h1t[j][:],
                                             start=(j == 0), stop=(j == 31))
                        fft = wpool.tile([128, SQ], BF16, tag="fft", bufs=2)
                        nc.vector.tensor_scalar_add(fft[:], ps[:],
                                                    b2_sb[:, i:i + 1])
                        ff_dst = ff_dl if i < 4 else ff_dh
                        nc.sync.dma_start(
                            ff_dst[(i % 4) * 128:(i % 4 + 1) * 128, :],
                            fft[:])
                        if i == 3:
                            emit_ffb(0, ffb_lo)
                            for sub in range(4):
                                y2 = l2pool.tile([128, D], F32, tag="y2",
                                                 bufs=4, name=f"y2_{sub}")
                                nc.gpsimd.tensor_add(
                                    y2[:, 0:512],
                                    x1_f32[sub][:, 0:512],
                                    ffb_lo[sub][:])
                                y2s.append(y2)

                # ---- Phase F: residual + LN2 + output ----
                ffb_hi = [None] * 4
                emit_ffb(1, ffb_hi)
                with tc.tile_pool(name="l2ps", bufs=2,
                                  space="PSUM") as l2ps:
                    for sub in range(4):
                        y2 = y2s[sub]
                        nc.gpsimd.tensor_add(
                            y2[:, 512:1024], x1_f32[sub][:, 512:1024],
                            ffb_hi[sub][:])
                        x2 = l2pool.tile([128, D], F32, tag="x2")
                        _layer_norm(nc, lpool, l2ps, y2, x2, eps_sb,
                                    (g2_sb, be2_sb) if apply_affine
                                    else None)
                        nc.sync.dma_start(
                            out_q[sub * 128:(sub + 1) * 128, :], x2[:])

    nc.compile()
    _BUILD_CACHE[apply_affine] = nc
    return nc


def _layer_norm(nc, pool, pspool, y, out, eps_sb, affine):
    """out = (y - mean(y)) * rsqrt(var(y) + EPS) [* g + b], free-dim D."""
    s1 = pool.tile([128, 1], F32, tag="ln_s1")
    nc.vector.reduce_sum(s1[:], y[:], axis=mybir.AxisListType.X)
    mean = pool.tile([128, 1], F32, tag="ln_mean")
    nc.vector.tensor_scalar_mul(mean[:], s1[:], 1.0 / D)
    cen = pool.tile([128, D], F32, tag="ln_cen")
    nc.vector.tensor_scalar_sub(cen[:], y[:], mean[:])
    scr = pspool.tile([128, D], F32, tag="ln_scr")
    var = pool.tile([128, 1], F32, tag="ln_var")
    nc.scalar.activation(scr[:], cen[:], AF.Square, accum_out=var[:])
    std = pool.tile([128, 1], F32, tag="ln_std")
    nc.scalar.activation(std[:], var[:], AF.Sqrt, bias=eps_sb[:, 0:1],
                         scale=1.0 / D)
    rstd = pool.tile([128, 1], F32, tag="ln_rstd")
    nc.vector.reciprocal(rstd[:], std[:])
    if affine is None:
        nc.vector.tensor_scalar_mul(out[:], cen[:], rstd[:])
    else:
        g_sb, b_sb = affine
        nc.vector.scalar_tensor_tensor(
            out[:], cen[:], rstd[:], g_sb[:], ALU.mult, ALU.mult)
        nc.vector.tensor_add(out[:], out[:], b_sb[:])


def kernel(x, Wq, bq, ln1_g, ln1_b, W1, b1, W2, b2, ln2_g, ln2_b):
    x = np.asarray(x, np.float32)
    bf = dt.np(BF16)
    trivial = (np.all(ln1_g == 1) and np.all(ln1_b == 0)
               and np.all(ln2_g == 1) and np.all(ln2_b == 0))
    nc = _build(apply_affine=not trivial)

    base = {
        "wq": np.ascontiguousarray(
            np.asarray(Wq, np.float32).transpose(1, 0, 2).reshape(D, D)
        ).astype(bf),
        "bq_r": np.ascontiguousarray(
            np.asarray(bq, np.float32).reshape(8, 128).T),
        "w1": np.asarray(W1, np.float32).astype(bf),
        "b1_r": np.ascontiguousarray(
            np.asarray(b1, np.float32).reshape(32, 128).T),
        "w2": np.asarray(W2, np.float32).astype(bf),
        "b2_r": np.ascontiguousarray(
            np.asarray(b2, np.float32).reshape(8, 128).T),
    }
    if not trivial:
        for name, v in (("g1d", ln1_g), ("be1d", ln1_b),
                        ("g2d", ln2_g), ("be2d", ln2_b)):
            base[name] = np.ascontiguousarray(
                np.broadcast_to(np.asarray(v, np.float32), (128, D)))

    in_maps = []
    for c in range(NCORES):
        b, t = divmod(c, 4)
        xb = np.concatenate([x[b, t * SQ:], x[b, :t * SQ]], axis=0)
        in_maps.append({
            **base,
            "x_bf": np.ascontiguousarray(xb).astype(bf),
            "x_q": np.ascontiguousarray(x[b, t * SQ:(t + 1) * SQ]),
        })

    import os
    trace = bool(int(os.environ.get("KERNEL_TRACE", "0")))
    kw = {}
    if trace:
        kw = dict(trace=True,
                  tmpdir=os.environ.get("KERNEL_TRACE_DIR") or None)
    res = run_bass_kernel_spmd(nc, in_maps, core_ids=list(range(NCORES)),
                               **kw)
    if trace:
        print(f"HW exec time: {res.exec_time_ns} ns  "
              f"(mean {res.mean_exec_time_ns}, "
              f"max core {res.max_exec_time_core_id})")
    out = np.empty((B, S, D), np.float32)
    for c in range(NCORES):
        b, t = divmod(c, 4)
        out[b, t * SQ:(t + 1) * SQ] = res.results[c]["out_q"]
    return out

